# revision 2
# baseline (speedup 1.0000x reference)
"""Causal self-attention on 8 trn2 NeuronCores — v2 (single-pass pipeline).

Sharding: DP4 (batch) x TP2 (head groups of 8). Core c -> batch c//2,
head group c%2. Each core computes qkv^T for its 512 channels, causal
attention for its 8 heads, and a partial projection. Host sums the two
partials per batch and transposes (kernel emits y^T).

v2 changes vs baseline:
- bf16 operands for every matmul (same PE rate as f32r, half DMA/SBUF,
  2x DVE on mask-muls).
- Single fused pass: for j in 0..3 emit A(slice j) -> B(q-block j) ->
  C(q-block j), so exp/copies overlap matmuls across phases.
- Diagonal narrowing: for the k-block on the diagonal (i = 4j+o) only
  q-columns [128o, 512) are computed/exp'd/PV'd (-15% attention work).
  Causal mask is one shared [128,128] lower-triangle multiply.
- Bias folding: qk bias fused into Act PSUM-drain copies (W_q, b_q
  pre-scaled by 1/sqrt(hd) on host); v bias added during the DVE V
  drain (broadcast tile); proj bias fused into Act proj drain.
- exp for both heads of a pair in one Act instruction (strided AP over
  the two 512-col halves of the score PSUM tile).
- reciprocal_approx_fast for softmax denominators.
Row-sums still come from a ones-column appended to V (65th row of the
PV accumulator); no max-subtraction (scores ~ N(0,1)).
"""
import sys

sys.path.insert(0, "/opt/trn_rl_repo")

import numpy as np

import concourse.bass as bass
import concourse.tile as tile
from concourse import bacc, mybir

f32 = mybir.dt.float32
bf16 = mybir.dt.bfloat16
AFT = mybir.ActivationFunctionType

N_CORES = 8
B, T, C = 4, 2048, 1024
H, HD = 16, 64            # total heads, head dim
HPC = 8                   # heads per core
CPC = 512                 # channels per core (q, k or v)
NC_T = C // 128           # 8 C-tiles (contraction)
SCALE = 1.0 / np.sqrt(HD)


DBG = {"strided_memset": True, "merged_exp": True, "fast_recip": True,
       "narrow_diag": True}


def build_nc(repeat: int = 1):
    nc = bacc.Bacc("TRN2", target_bir_lowering=False, debug=False,
                   num_devices=N_CORES)

    xb_d = nc.dram_tensor("xb", [C, T], bf16, kind="ExternalInput")
    wqkv_d = nc.dram_tensor("wqkv", [C, 3 * CPC], bf16, kind="ExternalInput")
    qkb_d = nc.dram_tensor("qkb", [128, 8], f32, kind="ExternalInput")
    vb_d = nc.dram_tensor("vb", [128, CPC], f32, kind="ExternalInput")
    wp_d = nc.dram_tensor("wp", [CPC, C], bf16, kind="ExternalInput")
    bp_d = nc.dram_tensor("bp", [128, 8], f32, kind="ExternalInput")
    mask_d = nc.dram_tensor("mask", [128, 128], bf16, kind="ExternalInput")
    yt_d = nc.dram_tensor("yT", [C, T], f32, kind="ExternalOutput")

    with tile.TileContext(nc) as tc:
        def body(_=None):
            _build_body(nc, tc, xb_d, wqkv_d, qkb_d, vb_d, wp_d, bp_d,
                        mask_d, yt_d)
        if repeat == 1:
            body()
        else:
            with tc.For_i(0, repeat, 1):
                body()
    nc.compile()
    return nc


def _build_body(nc, tc, xb_d, wqkv_d, qkb_d, vb_d, wp_d, bp_d, mask_d,
                yt_d):
    pers_cm = tc.tile_pool(name="pers", bufs=1)
    pers = pers_cm.__enter__()
    mask = pers.tile([128, 128], bf16, name="mask")
    nc.sync.dma_start(mask[:], mask_d.ap())
    qkb = pers.tile([128, 8], f32, name="qkb")
    nc.sync.dma_start(qkb[:], qkb_d.ap())
    vb = pers.tile([128, CPC], f32, name="vb")
    nc.sync.dma_start(vb[:], vb_d.ap())
    bp = pers.tile([128, 8], f32, name="bp")
    nc.sync.dma_start(bp[:], bp_d.ap())

    wqkv = [pers.tile([128, 3 * CPC], bf16, name=f"wqkv{ci}")
            for ci in range(NC_T)]
    for ci in range(NC_T):
        nc.sync.dma_start(wqkv[ci][:],
                          wqkv_d.ap()[128 * ci:128 * ci + 128, :])
    wp = [pers.tile([128, C], bf16, name=f"wp{i}") for i in range(4)]
    for ci in range(4):
        nc.sync.dma_start(wp[ci][:], wp_d.ap()[128 * ci:128 * ci + 128, :])

    # qkv^T: QT/KT [c=128 x 4 tiles, t=2048]; V natural + ones column
    qt = [pers.tile([128, T], bf16, name=f"qt{i}") for i in range(4)]
    kt = [pers.tile([128, T], bf16, name=f"kt{i}") for i in range(4)]
    vaug = [pers.tile([128, 8 * 65], bf16, name=f"vaug{i}") for i in range(16)]
    for i in range(16):
        # only the ones column (col 64 of each 65-group) needs init;
        # the V drain overwrites cols 0-63 every iteration
        if DBG["strided_memset"]:
            nc.gpsimd.memset(
                vaug[i][:].rearrange("p (h w) -> p h w", w=65)[:, :, 64:65],
                1.0)
        else:
            nc.gpsimd.memset(vaug[i][:], 1.0)

    xt_cm = tc.tile_pool(name="xt", bufs=18)
    xt_pool = xt_cm.__enter__()
    pt_cm = tc.tile_pool(name="pt", bufs=3)
    pt_pool = pt_cm.__enter__()
    rl_cm = tc.tile_pool(name="rl", bufs=3)
    rl_pool = rl_cm.__enter__()
    rlb_cm = tc.tile_pool(name="rlb", bufs=3)
    rlb_pool = rlb_cm.__enter__()
    ot_cm = tc.tile_pool(name="ot", bufs=8)
    ot_pool = ot_cm.__enter__()
    yt_cm = tc.tile_pool(name="yt", bufs=4)
    yt_pool = yt_cm.__enter__()
    ac_cm = tc.tile_pool(name="ac", bufs=2, space="PSUM")
    ac_pool = ac_cm.__enter__()
    pst_cm = tc.tile_pool(name="pst", bufs=2, space="PSUM")
    pst_pool = pst_cm.__enter__()
    pot_cm = tc.tile_pool(name="pot", bufs=2, space="PSUM")
    pot_pool = pot_cm.__enter__()

    # ---- filler groups: one PSUM tile-group each (A and C phases) ----
    def emit_a_group(j, g, xts):
        if g < 8:                      # Q tiles 0-3, K tiles 4-7
            ps = ac_pool.tile([128, 512], f32, name="ac")
            for ci in range(NC_T):
                nc.tensor.matmul(
                    ps[:], wqkv[ci][:, 128 * g:128 * g + 128], xts[ci][:],
                    start=(ci == 0), stop=(ci == NC_T - 1))
            dst = qt[g] if g < 4 else kt[g - 4]
            nc.scalar.activation(dst[:, 512 * j:512 * j + 512], ps[:],
                                 AFT.Identity, bias=qkb[:, g:g + 1])
        else:                          # V tiles
            tt = g - 8
            ti = 4 * j + tt
            ps = ac_pool.tile([128, 512], f32, name="ac")
            for ci in range(NC_T):
                nc.tensor.matmul(
                    ps[:], xts[ci][:, 128 * tt:128 * tt + 128],
                    wqkv[ci][:, 1024:1536],
                    start=(ci == 0), stop=(ci == NC_T - 1))
            dst = vaug[ti][:].rearrange("p (h w) -> p h w", w=65)[:, :, 0:64]
            nc.vector.tensor_add(
                dst, ps[:].rearrange("p (h w) -> p h w", w=64),
                vb[:].rearrange("p (h w) -> p h w", w=64))

    def emit_c_group(j, g, ots):
        ps = ac_pool.tile([128, 512], f32, name="ac")
        for ci in range(4):
            nc.tensor.matmul(
                ps[:], wp[ci][:, 128 * g:128 * g + 128], ots[ci][:],
                start=(ci == 0), stop=(ci == 3))
        yt = yt_pool.tile([128, 512], f32, name="yt")
        nc.scalar.activation(yt[:], ps[:], AFT.Identity,
                             bias=bp[:, g:g + 1])
        nc.sync.dma_start(
            yt_d.ap()[128 * g:128 * g + 128, 512 * j:512 * j + 512],
            yt[:])

    def a_fillers(j):
        xts = []
        for ci in range(NC_T):         # prefetch the x^T slice now
            xtt = xt_pool.tile([128, 512], bf16, name="xt")
            nc.sync.dma_start(
                xtt[:],
                xb_d.ap()[128 * ci:128 * ci + 128, 512 * j:512 * j + 512])
            xts.append(xtt)
        return [(lambda g=g, xts=xts, j=j: emit_a_group(j, g, xts))
                for g in range(12)]

    def c_fillers(j, ots):
        return [(lambda g=g, ots=ots, j=j: emit_c_group(j, g, ots))
                for g in range(8)]

    def emit_b(j, fillers):
        """Attention for q-block j; fillers paced between i-steps."""
        n_steps = 16 * (j + 1)
        step = 0
        emitted = 0
        ots = []
        for hp in range(4):            # head pairs (2hp, 2hp+1)
            pots = [pot_pool.tile([65, 512], f32, name="pot")
                    for _ in range(2)]
            for i in range(4 * j + 4):
                o = i - 4 * j          # >= 0 on the diagonal
                off = 128 * o if o > 0 else 0
                w = 512 - off
                st = pst_pool.tile([128, 1024], f32, name="pst")
                for hl in range(2):
                    rows = slice(64 * hl, 64 * hl + 64)
                    nc.tensor.matmul(
                        st[:, 512 * hl:512 * hl + w],
                        kt[hp][rows, 128 * i:128 * i + 128],
                        qt[hp][rows, 512 * j + off:512 * j + 512],
                        start=True, stop=True)
                ptile = pt_pool.tile([128, 1024], bf16, name="pt")
                if DBG["merged_exp"]:
                    stv = st[:].rearrange("p (h w) -> p h w", w=512)[:, :, 0:w]
                    ptv = ptile[:].rearrange("p (h w) -> p h w", w=512)[:, :, 0:w]
                    nc.scalar.activation(ptv, stv, AFT.Exp)
                else:
                    for hl in range(2):
                        nc.scalar.activation(
                            ptile[:, 512 * hl:512 * hl + w],
                            st[:, 512 * hl:512 * hl + w], AFT.Exp)
                if i >= 4 * j:         # diagonal: mask first 128 cols
                    for hl in range(2):
                        sub = ptile[:, 512 * hl:512 * hl + 128]
                        nc.vector.tensor_mul(sub, sub, mask[:])
                for hl in range(2):
                    h = 2 * hp + hl
                    nc.tensor.matmul(
                        pots[hl][:, off:512],
                        vaug[i][:, 65 * h:65 * h + 65],
                        ptile[:, 512 * hl:512 * hl + w],
                        start=(i == 0), stop=(i == 4 * j + 3),
                        skip_group_check=True)
                step += 1
                want = len(fillers) * step // n_steps
                while emitted < want:
                    fillers[emitted]()
                    emitted += 1
            # normalize -> O^T tile [128 d, 512 q] for this head pair
            ot_t = ot_pool.tile([128, 512], bf16, name="ot")
            for hl in range(2):
                rl = rl_pool.tile([1, 512], f32, name="rl")
                if DBG["fast_recip"]:
                    # reciprocal_approx_fast reads garbage from a PSUM /
                    # offset-64 source on HW: stage the row to SBUF first
                    rs = rl_pool.tile([1, 512], f32, name="rs")
                    nc.vector.tensor_copy(rs[:], pots[hl][64:65, :])
                    nc.vector.reciprocal_approx_fast(rl[:], rs[:])
                else:
                    nc.vector.reciprocal(rl[:], pots[hl][64:65, :])
                rlb = rlb_pool.tile([64, 512], f32, name="rlb")
                nc.gpsimd.partition_broadcast(rlb[:], rl[:])
                nc.vector.tensor_mul(ot_t[64 * hl:64 * hl + 64, :],
                                     pots[hl][0:64, :], rlb[:])
            ots.append(ot_t)
        while emitted < len(fillers):
            fillers[emitted]()
            emitted += 1
        return ots

    # ---- main schedule: A(0) | B(0)+A(1) | B(1)+C(0)+A(2) | ... | C(3)
    for f in a_fillers(0):
        f()
    ots_of = {}
    for j in range(4):
        fillers = []
        cf = c_fillers(j - 1, ots_of[j - 1]) if j > 0 else []
        af = a_fillers(j + 1) if j < 3 else []
        # alternate A/C so neither drains too late
        n = max(len(cf), len(af))
        for k in range(n):
            if k < len(af):
                fillers.append(af[k])
            if k < len(cf):
                fillers.append(cf[k])
        ots_of[j] = emit_b(j, fillers)
    for f in c_fillers(3, ots_of[3]):
        f()

    for cm in (pot_cm, pst_cm, ac_cm, yt_cm, ot_cm, rlb_cm, rl_cm, pt_cm,
               xt_cm, pers_cm):
        cm.__exit__(None, None, None)


def make_inputs(x, W_attn, b_attn, W_proj, b_proj):
    """Host-side sharding: per-core input dicts."""
    x = np.asarray(x, np.float32)
    W_attn = np.asarray(W_attn, np.float32)
    b_attn = np.asarray(b_attn, np.float32)
    W_proj = np.asarray(W_proj, np.float32)
    b_proj = np.asarray(b_proj, np.float32)

    # lower-triangle [128,128] block mask (k row kk visible to q col qq)
    kk = np.arange(128)[:, None]
    qq = np.arange(128)[None, :]
    mask128 = (kk <= qq).astype(np.float32)

    in_maps = []
    for core in range(N_CORES):
        b, g = divmod(core, 2)
        cols = np.concatenate([
            np.arange(CPC * g, CPC * g + CPC),
            C + np.arange(CPC * g, CPC * g + CPC),
            2 * C + np.arange(CPC * g, CPC * g + CPC)])
        wqkv = W_attn[:, cols].copy()
        wqkv[:, :CPC] *= SCALE                       # fold q-scale into W_q
        bq = b_attn[cols].copy()                     # [1536]
        bq[:CPC] *= SCALE
        qkb = np.ascontiguousarray(bq[:1024].reshape(8, 128).T)
        vb = np.broadcast_to(bq[1024:1536][None, :], (128, CPC))
        wp = np.ascontiguousarray(W_proj[CPC * g:CPC * g + CPC, :])
        bpv = (b_proj if g == 0 else np.zeros(C, np.float32))
        bpv = np.ascontiguousarray(bpv.reshape(8, 128).T)
        import ml_dtypes
        bf = ml_dtypes.bfloat16
        in_maps.append({
            "xb": np.ascontiguousarray(x[b].T).astype(bf),
            "wqkv": np.ascontiguousarray(wqkv).astype(bf),
            "qkb": qkb.astype(np.float32),
            "vb": np.ascontiguousarray(vb).astype(np.float32),
            "wp": wp.astype(bf),
            "bp": bpv.astype(np.float32),
            "mask": mask128.astype(bf),
        })
    return in_maps


def unshard(results):
    out = np.empty((B, T, C), np.float32)
    for b in range(B):
        yt = results[2 * b]["yT"] + results[2 * b + 1]["yT"]
        out[b] = yt.T
    return out


_nc_cache = {}


def kernel(x, W_attn, b_attn, W_proj, b_proj):
    from concourse.bass_utils import run_bass_kernel_spmd
    if "nc" not in _nc_cache:
        _nc_cache["nc"] = build_nc(repeat=1)
    nc = _nc_cache["nc"]
    in_maps = make_inputs(x, W_attn, b_attn, W_proj, b_proj)
    res = run_bass_kernel_spmd(nc, in_maps, core_ids=list(range(N_CORES)),
                               trace=False)
    return unshard(res.results)


# revision 3
# speedup vs baseline: 1.0019x; 1.0019x over previous
"""Causal self-attention on 8 trn2 NeuronCores — v3 (cross-iteration pipeline).

Sharding: DP4 (batch) x TP2 (head groups of 8). Core c -> batch c//2,
head group c%2. Each core computes qkv^T for its 512 channels, causal
attention for its 8 heads, and a partial projection. Host sums the two
partials per batch and transposes (kernel emits y^T).

v3 = v2 (bf16 operands, diagonal narrowing, fused biases, merged-hl
exp, staged fast reciprocal) plus a software pipeline ACROSS repeat
iterations: qt/kt/vaug are double-buffered (ping/pong sets) and the
whole A phase (qkv^T) of the NEXT iteration runs as filler groups
inside the attention drum of the CURRENT iteration, so the PE never
drains between iterations. The repeat loop runs the double body
(pass ping->pong, pass pong->ping) repeat/2 times.
"""
import sys

sys.path.insert(0, "/opt/trn_rl_repo")

import numpy as np

import concourse.bass as bass
import concourse.tile as tile
from concourse import bacc, mybir

f32 = mybir.dt.float32
bf16 = mybir.dt.bfloat16
AFT = mybir.ActivationFunctionType

N_CORES = 8
B, T, C = 4, 2048, 1024
H, HD = 16, 64
HPC = 8
CPC = 512
NC_T = C // 128
SCALE = 1.0 / np.sqrt(HD)


class _Ctx:
    pass


def build_nc(repeat: int = 1):
    assert repeat == 1 or repeat % 2 == 0, repeat
    nc = bacc.Bacc("TRN2", target_bir_lowering=False, debug=False,
                   num_devices=N_CORES)

    x = _Ctx()
    x.nc = nc
    x.xb_d = nc.dram_tensor("xb", [C, T], bf16, kind="ExternalInput")
    x.wqkv_d = nc.dram_tensor("wqkv", [C, 3 * CPC], bf16,
                              kind="ExternalInput")
    x.qkb_d = nc.dram_tensor("qkb", [128, 8], f32, kind="ExternalInput")
    x.vb_d = nc.dram_tensor("vb", [128, CPC], f32, kind="ExternalInput")
    x.wp_d = nc.dram_tensor("wp", [CPC, C], bf16, kind="ExternalInput")
    x.bp_d = nc.dram_tensor("bp", [128, 8], f32, kind="ExternalInput")
    x.mask_d = nc.dram_tensor("mask", [128, 128], bf16, kind="ExternalInput")
    x.yt_d = nc.dram_tensor("yT", [C, T], f32, kind="ExternalOutput")

    with tile.TileContext(nc) as tc:
        x.tc = tc
        cms = _setup(x)
        _emit_weight_dma(x)
        for j in range(4):             # prologue: fill set 0
            for f in _a_fillers(x, 0, j):
                f()
        if repeat == 1:
            carry = _emit_pass(x, 0, 1, [])
            for f in carry:
                f()
        else:
            with tc.For_i(0, repeat // 2, 1):
                c1 = _emit_pass(x, 0, 1, [])
                c2 = _emit_pass(x, 1, 0, c1)
                for f in c2:
                    f()
        for cm in cms:
            cm.__exit__(None, None, None)
    nc.compile()
    return nc


def _setup(x):
    nc, tc = x.nc, x.tc
    cms = []

    def pool(name, bufs, space="SBUF"):
        cm = tc.tile_pool(name=name, bufs=bufs, space=space)
        cms.insert(0, cm)
        return cm.__enter__()

    pers = pool("pers", 1)
    x.mask = pers.tile([128, 128], bf16, name="mask")
    x.qkb = pers.tile([128, 8], f32, name="qkb")
    x.vb = pers.tile([128, CPC], f32, name="vb")
    x.bp = pers.tile([128, 8], f32, name="bp")
    x.wqkv = [pers.tile([128, 3 * CPC], bf16, name=f"wqkv{ci}")
              for ci in range(NC_T)]
    x.wp = [pers.tile([128, C], bf16, name=f"wp{i}") for i in range(4)]
    # ping/pong qkv sets
    x.qt = [[pers.tile([128, T], bf16, name=f"qt{s}_{i}") for i in range(4)]
            for s in range(2)]
    x.kt = [[pers.tile([128, T], bf16, name=f"kt{s}_{i}") for i in range(4)]
            for s in range(2)]
    x.vaug = [[pers.tile([128, 8 * 65], bf16, name=f"vaug{s}_{i}")
               for i in range(16)] for s in range(2)]
    for s in range(2):
        for i in range(16):
            nc.gpsimd.memset(
                x.vaug[s][i][:].rearrange("p (h w) -> p h w", w=65)
                [:, :, 64:65], 1.0)

    x.xt_pool = pool("xt", 18)
    x.pt_pool = pool("pt", 4)
    x.rl_pool = pool("rl", 6)
    x.rlb_pool = pool("rlb", 3)
    x.ot_pool = pool("ot", 8)
    x.yt_pool = pool("yt", 4)
    x.ac_pool = pool("ac", 2, "PSUM")
    x.pst_pool = pool("pst", 2, "PSUM")
    x.pot_pool = pool("pot", 2, "PSUM")
    return cms


def _emit_weight_dma(x):
    nc = x.nc
    nc.sync.dma_start(x.mask[:], x.mask_d.ap())
    nc.sync.dma_start(x.qkb[:], x.qkb_d.ap())
    nc.sync.dma_start(x.vb[:], x.vb_d.ap())
    nc.sync.dma_start(x.bp[:], x.bp_d.ap())
    for ci in range(NC_T):
        nc.sync.dma_start(x.wqkv[ci][:],
                          x.wqkv_d.ap()[128 * ci:128 * ci + 128, :])
    for ci in range(4):
        nc.sync.dma_start(x.wp[ci][:],
                          x.wp_d.ap()[128 * ci:128 * ci + 128, :])


def _a_fillers(x, s, j):
    """12 filler groups computing qkv^T slice j into set s (+ x^T DMAs)."""
    nc = x.nc
    xts = []
    for ci in range(NC_T):
        xtt = x.xt_pool.tile([128, 512], bf16, name="xt")
        nc.sync.dma_start(
            xtt[:],
            x.xb_d.ap()[128 * ci:128 * ci + 128, 512 * j:512 * j + 512])
        xts.append(xtt)

    state = {}

    def emit(g, half):
        if g < 8:                      # Q tiles 0-3, K tiles 4-7
            if half == 0:
                state[g] = x.ac_pool.tile([128, 512], f32, name="ac")
            ps = state[g]
            for ci in (range(0, 4) if half == 0 else range(4, NC_T)):
                nc.tensor.matmul(
                    ps[:], x.wqkv[ci][:, 128 * g:128 * g + 128], xts[ci][:],
                    start=(ci == 0), stop=(ci == NC_T - 1))
            if half == 1:
                dst = x.qt[s][g] if g < 4 else x.kt[s][g - 4]
                nc.scalar.activation(dst[:, 512 * j:512 * j + 512], ps[:],
                                     AFT.Identity, bias=x.qkb[:, g:g + 1])
        else:                          # V tiles
            tt = g - 8
            ti = 4 * j + tt
            if half == 0:
                state[g] = x.ac_pool.tile([128, 512], f32, name="ac")
            ps = state[g]
            for ci in (range(0, 4) if half == 0 else range(4, NC_T)):
                nc.tensor.matmul(
                    ps[:], xts[ci][:, 128 * tt:128 * tt + 128],
                    x.wqkv[ci][:, 1024:1536],
                    start=(ci == 0), stop=(ci == NC_T - 1))
            if half == 1:
                dst = (x.vaug[s][ti][:]
                       .rearrange("p (h w) -> p h w", w=65)[:, :, 0:64])
                nc.vector.tensor_add(
                    dst, ps[:].rearrange("p (h w) -> p h w", w=64),
                    x.vb[:].rearrange("p (h w) -> p h w", w=64))

    return [(lambda g=g, h=h: emit(g, h))
            for g in range(12) for h in range(2)]


def _c_fillers(x, j, ots):
    """8 filler groups projecting q-block j from the given O^T tiles."""
    nc = x.nc

    state = {}

    def emit(g, half):
        if half == 0:
            state[g] = x.ac_pool.tile([128, 512], f32, name="ac")
        ps = state[g]
        for ci in (range(0, 2) if half == 0 else range(2, 4)):
            nc.tensor.matmul(
                ps[:], x.wp[ci][:, 128 * g:128 * g + 128], ots[ci][:],
                start=(ci == 0), stop=(ci == 3))
        if half == 1:
            yt = x.yt_pool.tile([128, 512], f32, name="yt")
            nc.scalar.activation(yt[:], ps[:], AFT.Identity,
                                 bias=x.bp[:, g:g + 1])
            nc.scalar.dma_start(
                x.yt_d.ap()[128 * g:128 * g + 128, 512 * j:512 * j + 512],
                yt[:])

    return [(lambda g=g, h=h: emit(g, h))
            for g in range(8) for h in range(2)]


def _emit_b(x, s, j, fillers):
    """Attention for q-block j reading set s; fillers paced between steps."""
    nc = x.nc
    n_steps = 16 * (j + 1)
    step = 0
    emitted = 0
    ots = []
    for hp in range(4):                # head pairs (2hp, 2hp+1)
        pots = [x.pot_pool.tile([65, 512], f32, name="pot")
                for _ in range(2)]

        def emit_pv(i, pt):
            o = i - 4 * j
            off = 128 * o if o > 0 else 0
            w = 512 - off
            for hl in range(2):
                h = 2 * hp + hl
                rhs = pt[:, 512 * hl:512 * hl + w]
                nc.tensor.matmul(
                    pots[hl][:, off:512],
                    x.vaug[s][i][:, 65 * h:65 * h + 65],
                    rhs,
                    start=(i == 0), stop=(i == 4 * j + 3),
                    skip_group_check=True)

        prev = None                    # one step behind: the PV pair runs
        for i in range(4 * j + 4):     # under the next S pair so the exp
            o = i - 4 * j              # latency is off PE's path
            off = 128 * o if o > 0 else 0
            w = 512 - off
            st = x.pst_pool.tile([128, 1024], f32, name="pst")
            for hl in range(2):
                rows = slice(64 * hl, 64 * hl + 64)
                nc.tensor.matmul(
                    st[:, 512 * hl:512 * hl + w],
                    x.kt[s][hp][rows, 128 * i:128 * i + 128],
                    x.qt[s][hp][rows, 512 * j + off:512 * j + 512],
                    start=True, stop=True)
            ptile = x.pt_pool.tile([128, 1024], bf16, name="pt")
            stv = st[:].rearrange("p (h w) -> p h w", w=512)[:, :, 0:w]
            ptv = ptile[:].rearrange("p (h w) -> p h w", w=512)[:, :, 0:w]
            nc.scalar.activation(ptv, stv, AFT.Exp)
            if i >= 4 * j:             # diagonal: mask first 128 cols
                for hl in range(2):
                    sub = ptile[:, 512 * hl:512 * hl + 128]
                    nc.vector.tensor_mul(sub, sub, x.mask[:])
            getp = ptile
            step += 1
            want = len(fillers) * step // n_steps
            while emitted < want:      # fillers run between S(i) and
                fillers[emitted]()     # PV(i-1): they widen the window
                emitted += 1           # that hides the exp latency
            if prev is not None:
                emit_pv(*prev)
            prev = (i, getp)
        emit_pv(*prev)
        # normalize -> O^T tile [128 d, 512 q] for this head pair
        ot_t = x.ot_pool.tile([128, 512], bf16, name="ot")
        for hl in range(2):
            rl = x.rl_pool.tile([1, 512], f32, name="rl")
            # reciprocal_approx_fast misreads PSUM/offset-64 sources on
            # HW: stage the row to SBUF first
            rs = x.rl_pool.tile([1, 512], f32, name="rs")
            nc.vector.tensor_copy(rs[:], pots[hl][64:65, :])
            nc.vector.reciprocal_approx_fast(rl[:], rs[:])
            rlb = x.rlb_pool.tile([64, 512], f32, name="rlb")
            nc.gpsimd.partition_broadcast(rlb[:], rl[:])
            nc.vector.tensor_mul(ot_t[64 * hl:64 * hl + 64, :],
                                 pots[hl][0:64, :], rlb[:])
        ots.append(ot_t)
    while emitted < len(fillers):
        fillers[emitted]()
        emitted += 1
    return ots


def _emit_pass(x, src, dst, carry_c):
    """One iteration's B/C reading set src; A fillers write set dst.
    Returns this pass's C(3) fillers (to interleave into the next pass)."""
    _emit_weight_dma(x)
    ots_of = {}
    for j in range(4):
        cf = carry_c if j == 0 else _c_fillers(x, j - 1, ots_of[j - 1])
        af = _a_fillers(x, dst, j)
        fillers = []
        for k in range(max(len(cf), len(af))):
            if k < len(af):
                fillers.append(af[k])
            if k < len(cf):
                fillers.append(cf[k])
        ots_of[j] = _emit_b(x, src, j, fillers)
    return _c_fillers(x, 3, ots_of[3])


def make_inputs(x, W_attn, b_attn, W_proj, b_proj):
    """Host-side sharding: per-core input dicts."""
    import ml_dtypes
    bf = ml_dtypes.bfloat16
    x = np.asarray(x, np.float32)
    W_attn = np.asarray(W_attn, np.float32)
    b_attn = np.asarray(b_attn, np.float32)
    W_proj = np.asarray(W_proj, np.float32)
    b_proj = np.asarray(b_proj, np.float32)

    kk = np.arange(128)[:, None]
    qq = np.arange(128)[None, :]
    mask128 = (kk <= qq).astype(np.float32)

    in_maps = []
    for core in range(N_CORES):
        b, g = divmod(core, 2)
        cols = np.concatenate([
            np.arange(CPC * g, CPC * g + CPC),
            C + np.arange(CPC * g, CPC * g + CPC),
            2 * C + np.arange(CPC * g, CPC * g + CPC)])
        wqkv = W_attn[:, cols].copy()
        wqkv[:, :CPC] *= SCALE                       # fold q-scale into W_q
        bq = b_attn[cols].copy()
        bq[:CPC] *= SCALE
        qkb = np.ascontiguousarray(bq[:1024].reshape(8, 128).T)
        vb = np.broadcast_to(bq[1024:1536][None, :], (128, CPC))
        wp = np.ascontiguousarray(W_proj[CPC * g:CPC * g + CPC, :])
        bpv = (b_proj if g == 0 else np.zeros(C, np.float32))
        bpv = np.ascontiguousarray(bpv.reshape(8, 128).T)
        in_maps.append({
            "xb": np.ascontiguousarray(x[b].T).astype(bf),
            "wqkv": np.ascontiguousarray(wqkv).astype(bf),
            "qkb": qkb.astype(np.float32),
            "vb": np.ascontiguousarray(vb).astype(np.float32),
            "wp": wp.astype(bf),
            "bp": bpv.astype(np.float32),
            "mask": mask128.astype(bf),
        })
    return in_maps


def unshard(results):
    out = np.empty((B, T, C), np.float32)
    for b in range(B):
        yt = results[2 * b]["yT"] + results[2 * b + 1]["yT"]
        out[b] = yt.T
    return out


_nc_cache = {}


def kernel(x, W_attn, b_attn, W_proj, b_proj):
    from concourse.bass_utils import run_bass_kernel_spmd
    if "nc" not in _nc_cache:
        _nc_cache["nc"] = build_nc(repeat=1)
    nc = _nc_cache["nc"]
    in_maps = make_inputs(x, W_attn, b_attn, W_proj, b_proj)
    res = run_bass_kernel_spmd(nc, in_maps, core_ids=list(range(N_CORES)),
                               trace=False)
    return unshard(res.results)
